# revision 37
# baseline (speedup 1.0000x reference)
"""Trainium2 Bass kernel for nn_CamMemory (soft cross-entropy vs. memory bank).

Computes: x = normalize(inputs); logits = x @ features.T / TEMP;
loss = mean_b( lse(logits_b) - dot(softmax(targets_b), logits_b) )

Sharding: features/targets split row-wise (N dim) across 8 cores; inputs
replicated.  Each core returns partial stats (s, p, u) per batch row:
  s = sum_n exp(logits - SHIFT)      (partial sum-exp, fixed shift; |logits|<=21)
  p = sum_n exp(targets - 1)*logits  (partial weighted logit sum)
  u = sum_n exp(targets - 1)         (partial softmax denominator; targets in [0,1))
Host combines: loss = mean_b( SHIFT + log(sum s) - (sum p)/(sum u) ).

Per-core schedule (wire = 16.8MB SWDGE cast-DMA of features at HBM rate,
~45us; everything else hides under it):
  - inputs/targets ride the two HWDGE rings (sync/scalar) as f32 during the
    SWDGE spin-up dead time; x-norm via ACT Square+Sqrt, scale+cast on ACT.
  - 17 feature pieces (15x128 + 2x64 rows; short tail pieces halve the
    drain): PE transpose-mode 128x128 blocks -> PSUM, DVE copies to SBUF
    featT; matmuls use featT blocks STATIONARY, xT (64 cols) MOVING ->
    logitsT [128n, 64b].  PE work for piece i-1's matmuls interleaves with
    piece i's transposes (one-piece software pipeline) so the PE never
    waits on the DVE copy it just enabled.
  - Epilogue (two-piece lag): ACT exp -> el, DVE etT*logits -> pm; s/p/u
    reduced over n by ones-matmuls ACCUMULATED in one PSUM bank across all
    pieces (disjoint 64-col ranges; per-element has_written semantics).
"""

import numpy as np

import concourse.bacc as bacc
import concourse.mybir as mybir
import concourse.tile as tile
from concourse.masks import make_identity

B = 64
D = 2048
N = 16384
NUM_CORES = 8
NSH = N // NUM_CORES  # 2048 rows of features per core
TEMP = 0.05
SHIFT = 21.0  # |logits| <= (1/TEMP)*|x.f| <= 20*(1+eps) since both unit-norm

F32 = mybir.dt.float32
BF16 = mybir.dt.bfloat16


def build_nc(d=D, nsh=NSH, b=B, debug=False):
    """Build the single-core Bass program (SPMD: same program, 8 shards)."""
    kc = d // 128     # contraction chunks (d on partitions)
    nch = nsh // 128  # feature-row chunks
    TG = 8            # transposed blocks staged per PSUM bank
    ngrp = kc // TG
    NWARM = 24

    nc = bacc.Bacc("TRN2", target_bir_lowering=False, debug=debug)

    # register extra const APs (framework style: preamble gpsimd memsets)
    for val in (-1.0, -float(SHIFT)):
        cten = nc.alloc_sbuf_tensor(f"const-f32-{val}", [128, 1], F32)
        nc.gpsimd.memset(cten.ap(), val)
        nc.const_aps.aps[(F32, val)] = cten.ap()

    inputs_d = nc.dram_tensor("inputs", [b, d], F32, kind="ExternalInput")
    targets_d = nc.dram_tensor("targets", [b, nsh], F32, kind="ExternalInput")
    features_d = nc.dram_tensor("features", [nsh, d], F32, kind="ExternalInput")
    out_d = nc.dram_tensor("out", [1, 192], F32, kind="ExternalOutput")

    with tile.TileContext(nc) as tc:
        with (
            tc.tile_pool(name="small", bufs=1) as small,
            tc.tile_pool(name="nat", bufs=16) as natp,
            tc.tile_pool(name="ft", bufs=4) as ftp,
            tc.tile_pool(name="epi", bufs=3) as epi,
            tc.tile_pool(name="tps", bufs=3, space="PSUM") as tpsp,
            tc.tile_pool(name="lps", bufs=3, space="PSUM") as lpsp,
            tc.tile_pool(name="spu", bufs=2, space="PSUM") as spup,
        ):
            # ---- x / targets on the HWDGE rings (parallel to SWDGE spin-up)
            xin = small.tile([b, d], F32)
            nc.sync.dma_start(xin[:], inputs_d[:])
            tg = small.tile([b, nsh], F32)
            nc.sync.dma_start(tg[:], targets_d[:])

            # ---- feature cast-DMAs: gpsimd issues these first.
            pieces = [(i * 128, 128) for i in range(nch)]
            natcs = []
            ident = identf = None
            for i, (r0, nr) in enumerate(pieces):
                natc = natp.tile([128, d], BF16, tag="nat")
                nc.gpsimd.dma_start(natc[0:nr, :], features_d[r0:r0 + nr, :])
                natcs.append(natc)
                if i == 1:
                    ident = small.tile([128, 128], BF16)
                    make_identity(nc, ident[:])
                elif i == 3:
                    identf = small.tile([b, b], F32)
                    make_identity(nc, identf[:])

            # ones on ACT from ident (keeps gpsimd free for DMA issues;
            # activation float biases become const APs automatically)
            ones = small.tile([128, 1], BF16)
            nc.scalar.activation(
                ones[:], ident[:, 0:1], mybir.ActivationFunctionType.Copy,
                bias=1.0, scale=0.0)

            # HAM pre-warm: throwaway matmuls while the first cast-DMAs are
            # in flight, so the PE clock gate is 8/8 for the real work.
            dwarm = lpsp.tile([128, 64], F32, tag="lp")
            for _ in range(NWARM):
                nc.tensor.matmul(dwarm[:], ident[:], ident[:, 0:64],
                                 start=True, stop=True)

            # ---- x norm chain (ACT-heavy; latency hides under DMA spin-up):
            # ss = sum x^2 (ACT Square+accum), srt = sqrt(T^2 ss), inv (DVE),
            # xb2 = bf16(x * inv) on ACT.
            sq = small.tile([b, d], F32)
            ss = small.tile([b, 1], F32)
            nc.scalar.activation(
                sq[:], xin[:], mybir.ActivationFunctionType.Square,
                accum_out=ss[:])
            srt = small.tile([b, 1], F32)
            nc.scalar.activation(
                srt[:], ss[:], mybir.ActivationFunctionType.Sqrt,
                scale=float(TEMP) * float(TEMP))
            inv = small.tile([b, 1], F32)
            nc.vector.reciprocal(inv[:], srt[:])
            xb2 = small.tile([b, d], BF16)
            nc.vector.tensor_scalar_mul(xb2[:], xin[:], inv[:])

            xT = small.tile([128, kc, 64], BF16)
            etT = small.tile([128, nch, b], BF16)

            def emit_xt():
                for g in range(ngrp):
                    tpx = tpsp.tile([128, TG, 128], BF16, tag="tps")
                    for j in range(TG):
                        k = g * TG + j
                        nc.tensor.transpose(
                            tpx[:, j, 0:b], xb2[:, k * 128:(k + 1) * 128],
                            ident[0:b, 0:b])
                    nc.vector.tensor_copy(xT[:, g * TG:(g + 1) * TG, :],
                                          tpx[:, :, 0:b])

            def emit_tt():
                # f32 transposes straight from tg; exp(t-1) fuses the
                # PSUM->SBUF move on ACT (no bf16 cast pass needed).
                for g in range(nch // TG):
                    tpt = tpsp.tile([128, TG, 64], F32, tag="tps")
                    for j in range(TG):
                        c = g * TG + j
                        nc.tensor.transpose(
                            tpt[:, j, :], tg[:, c * 128:(c + 1) * 128],
                            identf[:])
                    # et = exp(t) (the exp(t-1) offset cancels in p/u)
                    nc.scalar.activation(
                        etT[:, g * TG:(g + 1) * TG, :], tpt[:],
                        mybir.ActivationFunctionType.Exp)

            # ---- s/p/u: ones-matmuls accumulate across pieces into one
            # PSUM bank (3 disjoint col ranges -> 3 groups; per-element
            # has_written makes the interleaving safe).
            acc = small.tile([1, 192], F32)

            def emit_epi(prev, first, last):
                r0, nr, plps = prev
                ci, po = r0 // 128, r0 % 128
                pe = po + nr
                ets = etT[po:pe, ci, :]
                el = epi.tile([128, 64], BF16, tag="el")
                nc.scalar.activation(
                    el[po:pe, :], plps[po:pe, :],
                    mybir.ActivationFunctionType.Exp, bias=-float(SHIFT))
                pm = epi.tile([128, 64], BF16, tag="pm")
                nc.vector.tensor_mul(pm[po:pe, :], ets, plps[po:pe, :])
                spu = spup.tile([1, 192], F32, tag="spu")
                nc.tensor.matmul(spu[:, 0:64], ones[po:pe, :], el[po:pe, :],
                                 start=True, stop=True)
                nc.tensor.matmul(spu[:, 64:128], ones[po:pe, :], pm[po:pe, :],
                                 start=True, stop=True)
                nc.tensor.matmul(spu[:, 128:192], ones[po:pe, :], ets,
                                 start=True, stop=True)
                if first:
                    nc.vector.tensor_copy(acc[:], spu[:])
                else:
                    nc.vector.tensor_add(acc[:], acc[:], spu[:])

            def emit_mm(prev):
                r0, nr, ftc = prev
                po = r0 % 128
                lps = lpsp.tile([128, 64], F32, tag="lp")
                for k in range(kc):
                    nc.tensor.matmul(
                        lps[po:po + nr, :], ftc[:, k, 0:nr], xT[:, k, :],
                        start=(k == 0), stop=(k == kc - 1),
                    )
                return (r0, nr, lps)

            # ---- feature pipeline with one-piece matmul lag and two-piece
            # epilogue lag: PE order per piece i is
            #   transposes(i) | matmuls(i-1) | epilogue(i-2)
            # so matmuls consume copies that completed during the previous
            # piece's transposes, and epilogues consume ACT/DVE results that
            # completed during the previous piece's matmuls.
            np_ = len(pieces)
            mmq = None
            epq = None
            for idx, ((r0, nr), natc) in enumerate(
                    zip(pieces, natcs, strict=True)):
                last_piece = (idx == np_ - 1)
                ftc = ftp.tile([128, kc, 128], BF16, tag="ftc")
                if last_piece:
                    lps_l = lpsp.tile([128, 64], F32, tag="lp")
                for g in range(ngrp):
                    tp = tpsp.tile([128, TG, 128], BF16, tag="tps")
                    for j in range(TG):
                        k = g * TG + j
                        nc.tensor.transpose(
                            tp[:, j, 0:nr], natc[0:nr, k * 128:(k + 1) * 128],
                            ident[0:nr, 0:nr])
                    nc.vector.tensor_copy(ftc[:, g * TG:(g + 1) * TG, 0:nr],
                                          tp[:, :, 0:nr])
                    if last_piece and g == 0:
                        # drain the pipeline eagerly: piece 14's matmuls and
                        # piece 13's epilogue run inside piece 15's copy wait
                        if epq is not None:
                            emit_epi(epq, first=False, last=False)
                            epq = None
                        if mmq is not None:
                            epq = emit_mm(mmq)
                            mmq = None
                    if last_piece:
                        # per-group eager matmuls for the final piece
                        for k in range(g * TG, (g + 1) * TG):
                            nc.tensor.matmul(
                                lps_l[:, :], ftc[:, k, :], xT[:, k, :],
                                start=(k == 0), stop=(k == kc - 1),
                            )
                if last_piece:
                    continue
                if idx == 0:
                    emit_xt()
                elif idx == 1:
                    emit_tt()

                if epq is not None:
                    emit_epi(epq, first=(idx == 2), last=False)
                epq = None
                if mmq is not None:
                    epq = emit_mm(mmq)
                mmq = (r0, nr, ftc)
            emit_epi(epq, first=False, last=False)
            emit_epi((pieces[-1][0], pieces[-1][1], lps_l), first=False,
                     last=True)

            # ---- output
            nc.sync.dma_start(out_d[:], acc[:])

    nc.compile()
    return nc


_NC_CACHE = None


def _run(inputs, trace=False, **spmd_kwargs):
    global _NC_CACHE
    from concourse.bass_utils import run_bass_kernel_spmd

    x = np.ascontiguousarray(np.asarray(inputs["inputs"], dtype=np.float32))
    t = np.asarray(inputs["targets"], dtype=np.float32)
    f = np.asarray(inputs["features"], dtype=np.float32)
    # cid is unused by the reference computation.

    if _NC_CACHE is None:
        _NC_CACHE = build_nc(debug=False)
    nc = _NC_CACHE

    in_maps = []
    for c in range(NUM_CORES):
        in_maps.append({
            "inputs": x,
            "targets": np.ascontiguousarray(t[:, c * NSH:(c + 1) * NSH]),
            "features": np.ascontiguousarray(f[c * NSH:(c + 1) * NSH, :]),
        })

    res = run_bass_kernel_spmd(
        nc, in_maps, core_ids=list(range(NUM_CORES)), trace=trace, **spmd_kwargs)
    outs = np.stack([r["out"] for r in res.results])  # [8, 1, 192]

    outs64 = outs.astype(np.float64).reshape(NUM_CORES, 192)
    s = outs64[:, 0:64].sum(0)
    p = outs64[:, 64:128].sum(0)
    u = outs64[:, 128:192].sum(0)
    lse = SHIFT + np.log(s)
    loss = np.mean(lse - p / u)
    return np.float32(loss), res


def kernel(**inputs: np.ndarray) -> np.ndarray:
    loss, _ = _run(inputs)
    return np.asarray(loss, dtype=np.float32)


# revision 38
# speedup vs baseline: 1.0643x; 1.0643x over previous
"""Trainium2 Bass kernel for nn_CamMemory (soft cross-entropy vs. memory bank).

Computes: x = normalize(inputs); logits = x @ features.T / TEMP;
loss = mean_b( lse(logits_b) - dot(softmax(targets_b), logits_b) )

Sharding: features/targets split row-wise (N dim) across 8 cores; inputs
replicated.  Each core returns partial stats (s, p, u) per batch row:
  s = sum_n exp(logits - SHIFT)      (partial sum-exp, fixed shift; |logits|<=21)
  p = sum_n exp(targets - 1)*logits  (partial weighted logit sum)
  u = sum_n exp(targets - 1)         (partial softmax denominator; targets in [0,1))
Host combines: loss = mean_b( SHIFT + log(sum s) - (sum p)/(sum u) ).

Per-core schedule (wire = 16.8MB SWDGE cast-DMA of features at HBM rate,
~45us; everything else hides under it):
  - inputs/targets ride the two HWDGE rings (sync/scalar) as f32 during the
    SWDGE spin-up dead time; x-norm via ACT Square+Sqrt, scale+cast on ACT.
  - 17 feature pieces (15x128 + 2x64 rows; short tail pieces halve the
    drain): PE transpose-mode 128x128 blocks -> PSUM, DVE copies to SBUF
    featT; matmuls use featT blocks STATIONARY, xT (64 cols) MOVING ->
    logitsT [128n, 64b].  PE work for piece i-1's matmuls interleaves with
    piece i's transposes (one-piece software pipeline) so the PE never
    waits on the DVE copy it just enabled.
  - Epilogue (two-piece lag): ACT exp -> el, DVE etT*logits -> pm; s/p/u
    reduced over n by ones-matmuls ACCUMULATED in one PSUM bank across all
    pieces (disjoint 64-col ranges; per-element has_written semantics).
"""

import numpy as np

import concourse.bacc as bacc
import concourse.mybir as mybir
import concourse.tile as tile
from concourse.masks import make_identity

B = 64
D = 2048
N = 16384
NUM_CORES = 8
NSH = N // NUM_CORES  # 2048 rows of features per core
TEMP = 0.05
SHIFT = 21.0  # |logits| <= (1/TEMP)*|x.f| <= 20*(1+eps) since both unit-norm

F32 = mybir.dt.float32
BF16 = mybir.dt.bfloat16


def build_nc(d=D, nsh=NSH, b=B, debug=False):
    """Build the single-core Bass program (SPMD: same program, 8 shards)."""
    kc = d // 128     # contraction chunks (d on partitions)
    nch = nsh // 128  # feature-row chunks
    TG = 8            # transposed blocks staged per PSUM bank
    ngrp = kc // TG
    NWARM = 24

    nc = bacc.Bacc("TRN2", target_bir_lowering=False, debug=debug)

    # register extra const APs (framework style: preamble gpsimd memsets)
    for val in (-1.0, -float(SHIFT)):
        cten = nc.alloc_sbuf_tensor(f"const-f32-{val}", [128, 1], F32)
        nc.gpsimd.memset(cten.ap(), val)
        nc.const_aps.aps[(F32, val)] = cten.ap()

    inputs_d = nc.dram_tensor("inputs", [b, d], F32, kind="ExternalInput")
    targets_d = nc.dram_tensor("targets", [b, nsh], F32, kind="ExternalInput")
    features_d = nc.dram_tensor("features", [nsh, d], F32, kind="ExternalInput")
    out_d = nc.dram_tensor("out", [1, 192], F32, kind="ExternalOutput")

    with tile.TileContext(nc) as tc:
        with (
            tc.tile_pool(name="small", bufs=1) as small,
            tc.tile_pool(name="nat", bufs=16) as natp,
            tc.tile_pool(name="ft", bufs=4) as ftp,
            tc.tile_pool(name="epi", bufs=3) as epi,
            tc.tile_pool(name="tps", bufs=3, space="PSUM") as tpsp,
            tc.tile_pool(name="lps", bufs=3, space="PSUM") as lpsp,
            tc.tile_pool(name="spu", bufs=2, space="PSUM") as spup,
        ):
            # ---- x / targets on the HWDGE rings (parallel to SWDGE spin-up)
            xin = small.tile([b, d], F32)
            nc.sync.dma_start(xin[:], inputs_d[:])
            tg = small.tile([b, nsh], F32)
            nc.sync.dma_start(tg[:], targets_d[:])

            # ---- feature cast-DMAs: gpsimd issues these first.
            pieces = [(i * 128, 128) for i in range(nch)]
            natcs = []
            ident = identf = None
            for i, (r0, nr) in enumerate(pieces):
                natc = natp.tile([128, d], BF16, tag="nat")
                nc.gpsimd.dma_start(natc[0:nr, :], features_d[r0:r0 + nr, :])
                natcs.append(natc)
                if i == 1:
                    ident = small.tile([128, 128], BF16)
                    make_identity(nc, ident[:])
                elif i == 3:
                    identf = small.tile([b, b], F32)
                    make_identity(nc, identf[:])

            # ones on ACT from ident (keeps gpsimd free for DMA issues;
            # activation float biases become const APs automatically)
            ones = small.tile([128, 1], BF16)
            nc.scalar.activation(
                ones[:], ident[:, 0:1], mybir.ActivationFunctionType.Copy,
                bias=1.0, scale=0.0)

            # HAM pre-warm: throwaway matmuls while the first cast-DMAs are
            # in flight, so the PE clock gate is 8/8 for the real work.
            dwarm = lpsp.tile([128, 64], F32, tag="lp")
            for _ in range(NWARM):
                nc.tensor.matmul(dwarm[:], ident[:], ident[:, 0:64],
                                 start=True, stop=True)

            # ---- x norm chain (ACT-heavy; latency hides under DMA spin-up):
            # ss = sum x^2 (ACT Square+accum), srt = sqrt(T^2 ss), inv (DVE),
            # xb2 = bf16(x * inv) on ACT.
            sq = small.tile([b, d], F32)
            ss = small.tile([b, 1], F32)
            nc.scalar.activation(
                sq[:], xin[:], mybir.ActivationFunctionType.Square,
                accum_out=ss[:])
            srt = small.tile([b, 1], F32)
            nc.scalar.activation(
                srt[:], ss[:], mybir.ActivationFunctionType.Sqrt,
                scale=float(TEMP) * float(TEMP))
            inv = small.tile([b, 1], F32)
            nc.vector.reciprocal(inv[:], srt[:])
            xb2 = small.tile([b, d], BF16)
            nc.vector.tensor_scalar_mul(xb2[:], xin[:], inv[:])

            xT = small.tile([128, kc, 64], BF16)
            etT = small.tile([128, nch, b], BF16)

            def emit_xt():
                for g in range(ngrp):
                    tpx = tpsp.tile([128, TG, 128], BF16, tag="tps")
                    for j in range(TG):
                        k = g * TG + j
                        nc.tensor.transpose(
                            tpx[:, j, 0:b], xb2[:, k * 128:(k + 1) * 128],
                            ident[0:b, 0:b])
                    nc.vector.tensor_copy(xT[:, g * TG:(g + 1) * TG, :],
                                          tpx[:, :, 0:b])

            def emit_tt():
                # f32 transposes straight from tg; exp(t-1) fuses the
                # PSUM->SBUF move on ACT (no bf16 cast pass needed).
                for g in range(nch // TG):
                    tpt = tpsp.tile([128, TG, 64], F32, tag="tps")
                    for j in range(TG):
                        c = g * TG + j
                        nc.tensor.transpose(
                            tpt[:, j, :], tg[:, c * 128:(c + 1) * 128],
                            identf[:])
                    # et = exp(t) (the exp(t-1) offset cancels in p/u)
                    nc.scalar.activation(
                        etT[:, g * TG:(g + 1) * TG, :], tpt[:],
                        mybir.ActivationFunctionType.Exp)

            # ---- s/p/u: ones-matmuls accumulate across pieces into one
            # PSUM bank (3 disjoint col ranges -> 3 groups; per-element
            # has_written makes the interleaving safe).
            acc = small.tile([1, 192], F32)

            def emit_epi(prev, first, last):
                r0, nr, plps = prev
                ci, po = r0 // 128, r0 % 128
                pe = po + nr
                ets = etT[po:pe, ci, :]
                el = epi.tile([128, 64], BF16, tag="el")
                nc.scalar.activation(
                    el[po:pe, :], plps[po:pe, :],
                    mybir.ActivationFunctionType.Exp, bias=-float(SHIFT))
                pm = epi.tile([128, 64], BF16, tag="pm")
                nc.vector.tensor_mul(pm[po:pe, :], ets, plps[po:pe, :])
                spu = spup.tile([1, 192], F32, tag="spu")
                nc.tensor.matmul(spu[:, 0:64], ones[po:pe, :], el[po:pe, :],
                                 start=True, stop=True)
                nc.tensor.matmul(spu[:, 64:128], ones[po:pe, :], pm[po:pe, :],
                                 start=True, stop=True)
                nc.tensor.matmul(spu[:, 128:192], ones[po:pe, :], ets,
                                 start=True, stop=True)
                if first:
                    nc.vector.tensor_copy(acc[:], spu[:])
                else:
                    nc.vector.tensor_add(acc[:], acc[:], spu[:])

            def emit_mm(prev):
                r0, nr, ftc = prev
                po = r0 % 128
                lps = lpsp.tile([128, 64], F32, tag="lp")
                for k in range(kc):
                    nc.tensor.matmul(
                        lps[po:po + nr, :], ftc[:, k, 0:nr], xT[:, k, :],
                        start=(k == 0), stop=(k == kc - 1),
                    )
                return (r0, nr, lps)

            # ---- feature pipeline with one-piece matmul lag and two-piece
            # epilogue lag: PE order per piece i is
            #   transposes(i) | matmuls(i-1) | epilogue(i-2)
            # so matmuls consume copies that completed during the previous
            # piece's transposes, and epilogues consume ACT/DVE results that
            # completed during the previous piece's matmuls.
            np_ = len(pieces)
            mmq = None
            epq = None
            for idx, ((r0, nr), natc) in enumerate(
                    zip(pieces, natcs, strict=True)):
                last_piece = (idx == np_ - 1)
                ftc = ftp.tile([128, kc, 128], BF16, tag="ftc")
                if last_piece:
                    lps_l = lpsp.tile([128, 64], F32, tag="lp")
                for g in range(ngrp):
                    tp = tpsp.tile([128, TG, 128], BF16, tag="tps")
                    for j in range(TG):
                        k = g * TG + j
                        nc.tensor.transpose(
                            tp[:, j, 0:nr], natc[0:nr, k * 128:(k + 1) * 128],
                            ident[0:nr, 0:nr])
                    nc.vector.tensor_copy(ftc[:, g * TG:(g + 1) * TG, 0:nr],
                                          tp[:, :, 0:nr])
                    if last_piece and g == 0:
                        # drain the pipeline eagerly: piece 14's matmuls and
                        # piece 13's epilogue run inside piece 15's copy wait
                        if epq is not None:
                            emit_epi(epq, first=False, last=False)
                            epq = None
                        if mmq is not None:
                            epq = emit_mm(mmq)
                            mmq = None
                if last_piece:
                    # eager matmuls for the final piece, right after its
                    # transposes (copies complete during MM(14) above)
                    for k in range(kc):
                        nc.tensor.matmul(
                            lps_l[:, :], ftc[:, k, :], xT[:, k, :],
                            start=(k == 0), stop=(k == kc - 1),
                        )
                    continue
                if idx == 0:
                    emit_xt()
                elif idx == 1:
                    emit_tt()

                if epq is not None:
                    emit_epi(epq, first=(idx == 2), last=False)
                epq = None
                if mmq is not None:
                    epq = emit_mm(mmq)
                mmq = (r0, nr, ftc)
            emit_epi(epq, first=False, last=False)
            emit_epi((pieces[-1][0], pieces[-1][1], lps_l), first=False,
                     last=True)

            # ---- output
            nc.sync.dma_start(out_d[:], acc[:])

    nc.compile()
    return nc


_NC_CACHE = None


def _run(inputs, trace=False, **spmd_kwargs):
    global _NC_CACHE
    from concourse.bass_utils import run_bass_kernel_spmd

    x = np.ascontiguousarray(np.asarray(inputs["inputs"], dtype=np.float32))
    t = np.asarray(inputs["targets"], dtype=np.float32)
    f = np.asarray(inputs["features"], dtype=np.float32)
    # cid is unused by the reference computation.

    if _NC_CACHE is None:
        _NC_CACHE = build_nc(debug=False)
    nc = _NC_CACHE

    in_maps = []
    for c in range(NUM_CORES):
        in_maps.append({
            "inputs": x,
            "targets": np.ascontiguousarray(t[:, c * NSH:(c + 1) * NSH]),
            "features": np.ascontiguousarray(f[c * NSH:(c + 1) * NSH, :]),
        })

    res = run_bass_kernel_spmd(
        nc, in_maps, core_ids=list(range(NUM_CORES)), trace=trace, **spmd_kwargs)
    outs = np.stack([r["out"] for r in res.results])  # [8, 1, 192]

    outs64 = outs.astype(np.float64).reshape(NUM_CORES, 192)
    s = outs64[:, 0:64].sum(0)
    p = outs64[:, 64:128].sum(0)
    u = outs64[:, 128:192].sum(0)
    lse = SHIFT + np.log(s)
    loss = np.mean(lse - p / u)
    return np.float32(loss), res


def kernel(**inputs: np.ndarray) -> np.ndarray:
    loss, _ = _run(inputs)
    return np.asarray(loss, dtype=np.float32)
